# revision 13
# baseline (speedup 1.0000x reference)
"""ColBERT-style max-sim retrieval kernel for 8 trn2 NeuronCores.

Computes, for query_h [Bq=128, Lq=32, H=256], doc_h [Bd=128, Ld=128, H=256],
W [256, 128], b [128]:

    q = l2norm(query_h @ W + b)          # [Bq, Lq, D=128]
    d = l2norm(doc_h  @ W + b)           # [Bd, Ld, D]
    logits[q, b] = sum_s max_t <q[q,s], d[b,t]>    # [Bq, Bd]

Sharding: docs split 8 x 16 across cores (queries replicated); each core
computes a [128, 16] column block of the logits; host concatenates.

Key structure (vs the 90.6us baseline, which was DVE-reduce-bound at 87%):
  - Everything ships/computes in bf16 (1 cycle/row matmuls, half the DMA).
  - Query normalization is algebraically moved past the max:
        sum_s max_t <q_s, d_t>/|q_s| = sum_s (1/|q_s|) max_t <q_s, d_t>
    so the q-side rrep/normalize (full [128, 4096] elementwise work)
    becomes a broadcast multiply on the tiny [128, 32, 16] max-values.
    |q_s|^2 comes from tiny per-tile PE matmuls (sq-tile @ ones-column)
    that land per-token sums on PARTITIONS (matching the score layout).
  - The max over doc tokens runs as a tensor_tensor max TREE instead of
    reduce_max: InstTensorReduce has no DVE fast modes (1 elem/lane/cyc)
    while TT-max consumes 2 inputs/lane/cycle and gets a further 2x on
    packed bf16 in SBUF. Level 1 (PSUM fp32 -> SBUF bf16) is split across
    Pool (~7/8, 427ns/half) and DVE (~1/8, 658ns/half); levels 2..7 run
    batched per 4-tile group ([128, 64, 64] -> [128, 64]) on DVE bf16.
  - Doc embeddings are normalized as in the baseline (the 1/|d_t| scale
    sits inside the max and cannot move out).
"""

import sys

import numpy as np

if "/opt/trn_rl_repo" not in sys.path:
    sys.path.insert(0, "/opt/trn_rl_repo")

import concourse.bass as bass
import concourse.tile as tile
from concourse import bacc, mybir
from concourse.bass_utils import run_bass_kernel_spmd

F32 = mybir.dt.float32
F32R = mybir.dt.float32r
BF16 = mybir.dt.bfloat16
AX = mybir.AxisListType
ALU = mybir.AluOpType
ACTF = mybir.ActivationFunctionType

# Problem constants (hardcoded per the harness contract).
BQ, LQ, BD, LD, H, D = 128, 32, 128, 128, 256, 128
NCORES = 8
DOCS_PER_CORE = BD // NCORES          # 16
NQ_TOK = BQ * LQ                      # 4096 query tokens (replicated)
ND_TOK = DOCS_PER_CORE * LD           # 2048 doc tokens per core
CHUNK = 512                           # embedding-phase token chunk (1 psum bank)
QS_TILES = NQ_TOK // 128              # 32 score row-tiles
GQ = 128 // LQ                        # 4 queries per qs-tile
NGROUPS = QS_TILES // 4               # 8 tail-tree groups (4 tiles each)

# consts layout (bf16): W0 | W1 | ones | gpad
NCONST = 128 + 128 + 128 + 256


def _build_program() -> bass.Bass:
    # Bacc (not plain Bass): its compile() runs move_matmul_waits_to_ldweights
    # and generate_event_semaphores, which split multi-wait matmuls into
    # event-semaphore helpers -- walrus rejects a fused matmul with >1 wait.
    nc = bacc.Bacc("TRN2", target_bir_lowering=False)

    # Inputs: both H-halves packed per tensor so one DMA delivers a chunk.
    qhT = nc.dram_tensor("qhT", [128, 2, NQ_TOK], BF16, kind="ExternalInput")
    dhT = nc.dram_tensor("dhT", [128, 2, ND_TOK], BF16, kind="ExternalInput")
    consts = nc.dram_tensor("consts", [128, NCONST], BF16, kind="ExternalInput")
    bb = nc.dram_tensor("bb", [128, 1], F32, kind="ExternalInput")
    out_d = nc.dram_tensor("logits", [128, DOCS_PER_CORE], F32, kind="ExternalOutput")

    with tile.TileContext(nc) as tc:
        with (
            tc.tile_pool(name="consts", bufs=1) as constp,
            tc.tile_pool(name="inputs", bufs=1) as inp,
            tc.tile_pool(name="embs", bufs=1) as embp,
        ):
            consts_sb = constp.tile([128, NCONST], BF16)
            b_sb = constp.tile([128, 1], F32)
            nc.sync.dma_start(consts_sb[:], consts[:])
            nc.sync.dma_start(b_sb[:], bb[:])
            w0_sb = consts_sb[:, 0:128]
            w1_sb = consts_sb[:, 128:256]
            ones_sb = consts_sb[:, 256:384]
            gpad_sb = consts_sb[:, 384:640]

            dhT_sb = inp.tile([128, 2, ND_TOK], BF16)
            qhT_sb = inp.tile([128, 2, NQ_TOK], BF16)
            # Docs first (they gate every score); q0 rides between d1 and d2.
            for c in range(0, 2 * CHUNK, CHUNK):
                nc.sync.dma_start(dhT_sb[:, :, c : c + CHUNK], dhT[:, :, c : c + CHUNK])
            nc.sync.dma_start(qhT_sb[:, :, 0:CHUNK], qhT[:, :, 0:CHUNK])
            for c in range(2 * CHUNK, ND_TOK, CHUNK):
                nc.sync.dma_start(dhT_sb[:, :, c : c + CHUNK], dhT[:, :, c : c + CHUNK])
            for c in range(CHUNK, NQ_TOK, CHUNK):
                nc.sync.dma_start(qhT_sb[:, :, c : c + CHUNK], qhT[:, :, c : c + CHUNK])

            embq = embp.tile([128, NQ_TOK], BF16)   # q emb + b, UNnormalized [D, tok]
            embd = embp.tile([128, ND_TOK], BF16)   # normalized d emb [D, tok]
            mvbuf = embp.tile([128, QS_TILES, DOCS_PER_CORE], BF16)  # raw maxes
            mvsc = embp.tile([128, QS_TILES, DOCS_PER_CORE], BF16)   # rq-scaled
            rq = embp.tile([128, QS_TILES], F32R)   # 1/|q_s| per tile-token

            with (
                tc.tile_pool(name="pe_psum", bufs=1, space="PSUM") as pep,
                tc.tile_pool(name="sc_psum", bufs=2, space="PSUM") as scp,
                tc.tile_pool(name="ss_psum", bufs=1, space="PSUM") as ssp,
                tc.tile_pool(name="sq_psum", bufs=1, space="PSUM") as sqp,
                tc.tile_pool(name="lg_psum", bufs=1, space="PSUM") as lgp,
                tc.tile_pool(name="actwork", bufs=2) as actp,
                tc.tile_pool(name="tails", bufs=2) as tailp,
                tc.tile_pool(name="outp", bufs=1) as outp,
            ):
                # Absorb DMA sem waits ahead of fused matmuls (single-wait
                # restriction): a tiny self-referencing observer matmul.
                def pe_observe(x):
                    ob = pep.tile([1, 2], F32, tag="pe")
                    nc.tensor.matmul(
                        ob[:], x[:, 0:1], x[:, 0:2], start=True, stop=True
                    )

                pe_observe(consts_sb)

                # First activation = Abs_reciprocal_sqrt so the table-load
                # pass picks abs_reciprocal_sqrt_and_small (contains square,
                # copy, identity too): exactly one ACT_TABLE_LOAD overall.
                act_seed = actp.tile([128, 1], F32, tag="seed", bufs=1)
                nc.scalar.activation(
                    act_seed[:], b_sb[:, 0:1], ACTF.Abs_reciprocal_sqrt
                )

                # per-token |q|^2 accumulator, one column per qs-tile
                ssq_ps = sqp.tile([128, QS_TILES], F32)

                def emb_doc(ci):
                    """Project+normalize doc tokens [ci*CHUNK, ...) -> embd."""
                    c = ci * CHUNK
                    pe = pep.tile([128, CHUNK], F32, tag="pe")
                    nc.tensor.matmul(
                        pe[:], w0_sb[:], dhT_sb[:, 0, c : c + CHUNK],
                        start=True, stop=False,
                    )
                    nc.tensor.matmul(
                        pe[:], w1_sb[:], dhT_sb[:, 1, c : c + CHUNK],
                        start=False, stop=True,
                    )
                    sq = actp.tile([128, CHUNK], BF16, tag="sq")
                    nc.scalar.activation(sq[:], pe[:], ACTF.Square, bias=b_sb[:])
                    # Cross-partition sum of squares broadcast to all rows.
                    ss = ssp.tile([128, CHUNK], F32, tag="ss")
                    nc.tensor.matmul(ss[:], ones_sb[:], sq[:], start=True, stop=True)
                    rrep = actp.tile([128, CHUNK], F32R, tag="rrep")
                    nc.scalar.activation(rrep[:], ss[:], ACTF.Abs_reciprocal_sqrt)
                    # embd = (pe + b) * rrep  (bf16 on write)
                    nc.vector.scalar_tensor_tensor(
                        out=embd[:, c : c + CHUNK],
                        in0=pe[:],
                        scalar=b_sb[:],
                        in1=rrep[:],
                        op0=ALU.add,
                        op1=ALU.mult,
                    )

                def emb_q(ci):
                    """Project q chunk ci; emit embq (+b), sqq, ssq cols, rq."""
                    c = ci * CHUNK
                    pe = pep.tile([128, CHUNK], F32, tag="pe")
                    nc.tensor.matmul(
                        pe[:], w0_sb[:], qhT_sb[:, 0, c : c + CHUNK],
                        start=True, stop=False,
                    )
                    nc.tensor.matmul(
                        pe[:], w1_sb[:], qhT_sb[:, 1, c : c + CHUNK],
                        start=False, stop=True,
                    )
                    nc.scalar.activation(
                        embq[:, c : c + CHUNK], pe[:], ACTF.Identity, bias=b_sb[:]
                    )
                    # sqq = embq^2 on Pool (SBUF-only mult is legal there;
                    # frees ACT for the PSUM score-copy pipeline).
                    sqq = actp.tile([128, CHUNK], BF16, tag="sqq")
                    nc.gpsimd.tensor_tensor(
                        out=sqq[:],
                        in0=embq[:, c : c + CHUNK],
                        in1=embq[:, c : c + CHUNK],
                        op=ALU.mult,
                    )
                    # Per-token sum over D (partitions): land tokens on
                    # partitions via tiny matmuls, one column per qs-tile.
                    # One accumulation-group start per chunk: a second
                    # start=True in the same bank re-marks the whole 2KB
                    # zero-region, which would pending-zero the earlier
                    # columns on hardware before the rsqrt reads them.
                    for j in range(4):
                        i = 4 * ci + j
                        nc.tensor.matmul(
                            ssq_ps[:, i : i + 1],
                            sqq[:, j * 128 : (j + 1) * 128],
                            ones_sb[:, 0:1],
                            start=(j == 0),
                            stop=(j == 3),
                            skip_group_check=True,
                        )
                    nc.scalar.activation(
                        rq[:, 4 * ci : 4 * ci + 4],
                        ssq_ps[:, 4 * ci : 4 * ci + 4],
                        ACTF.Abs_reciprocal_sqrt,
                    )

                def score_half(i, h, stage, c_idx):
                    """Scores for qs-tile i, docs [8h, 8h+8).

                    No engine may read two non-scalar PSUM operands in one
                    instruction (and Pool has no max op at all), so the
                    [128, 8, 128] PSUM scores leave PSUM through one of the
                    two legal 1-elem/lane/cycle consumers:
                      - R-halves: DVE reduce_max straight to mvbuf
                      - C-halves: ACT Copy -> bf16 stage; DVE then runs the
                        batched bf16 max tree at 2 max-ops/lane/cycle.
                    """
                    qsl = embq[:, i * 128 : (i + 1) * 128]
                    sc = scp.tile([128, 8, 128], F32, tag="sc")
                    for j in range(2):
                        col = h * 1024 + j * 512
                        nc.tensor.matmul(
                            sc[:, j * 4 : (j + 1) * 4, :],
                            qsl,
                            embd[:, col : col + 512],
                            start=True,
                            stop=True,
                        )
                    if c_idx is None:
                        nc.vector.reduce_max(
                            mvbuf[:, i, h * 8 : (h + 1) * 8], sc[:], axis=AX.X
                        )
                    else:
                        nc.scalar.activation(
                            stage[:, c_idx * 8 : c_idx * 8 + 8, :], sc[:],
                            ACTF.Copy,
                        )

                def tail_tree(g, stage, n_c):
                    """DVE bf16 max tree over the n_c ACT-copied halves of
                    group g -> the tail of mvbuf's flat [tile, doc] range."""
                    m = n_c * 8
                    t1 = tailp.tile([128, m, 64], BF16, tag="t1")
                    nc.vector.tensor_tensor(
                        out=t1[:], in0=stage[:, :, 0:64], in1=stage[:, :, 64:128],
                        op=ALU.max,
                    )
                    t2 = tailp.tile([128, m, 32], BF16, tag="t2")
                    nc.vector.tensor_tensor(
                        out=t2[:], in0=t1[:, :, 0:32], in1=t1[:, :, 32:64], op=ALU.max
                    )
                    t3 = tailp.tile([128, m, 16], BF16, tag="t3")
                    nc.vector.tensor_tensor(
                        out=t3[:], in0=t2[:, :, 0:16], in1=t2[:, :, 16:32], op=ALU.max
                    )
                    t4 = tailp.tile([128, m, 8], BF16, tag="t4")
                    nc.vector.tensor_tensor(
                        out=t4[:], in0=t3[:, :, 0:8], in1=t3[:, :, 8:16], op=ALU.max
                    )
                    t5 = tailp.tile([128, m, 4], BF16, tag="t5")
                    nc.vector.tensor_tensor(
                        out=t5[:], in0=t4[:, :, 0:4], in1=t4[:, :, 4:8], op=ALU.max
                    )
                    t6 = tailp.tile([128, m, 2], BF16, tag="t6")
                    nc.vector.tensor_tensor(
                        out=t6[:], in0=t5[:, :, 0:2], in1=t5[:, :, 2:4], op=ALU.max
                    )
                    start = (4 * g + 4) * 16 - m
                    mv = mvbuf[:, :, :].rearrange("p a b -> p (a b)")[
                        :, start : start + m
                    ].unsqueeze(2)
                    nc.vector.tensor_tensor(
                        out=mv, in0=t6[:, :, 0:1], in1=t6[:, :, 1:2], op=ALU.max
                    )

                def apply_rq(g):
                    rqb = rq[:, 4 * g : 4 * g + 4, None].broadcast_to(
                        [128, 4, DOCS_PER_CORE]
                    )
                    nc.vector.tensor_tensor(
                        out=mvsc[:, 4 * g : 4 * g + 4, :],
                        in0=mvbuf[:, 4 * g : 4 * g + 4, :],
                        in1=rqb,
                        op=ALU.mult,
                    )

                def group_sums(g, logits_ps):
                    # Sum over the 32 tokens of each query via a sliding
                    # block-diagonal window of gpad (weights are 0/1).
                    for i in range(4 * g, 4 * g + 4):
                        off = 124 - GQ * i
                        nc.tensor.matmul(
                            logits_ps[:],
                            gpad_sb[:, off : off + 128],
                            mvsc[:, i, :],
                            start=(i == 0),
                            stop=(i == QS_TILES - 1),
                            skip_group_check=True,
                        )

                # --- ramp ---
                emb_doc(0)
                emb_doc(1)
                emb_q(0)
                emb_doc(2)
                emb_q(1)
                emb_doc(3)

                logits_ps = lgp.tile([128, DOCS_PER_CORE], F32)

                # --- steady state: 8 groups of 4 qs-tiles ---
                # R/C split per group (R-halves head, C-halves tail of the
                # group's flat [tile, h] order so the tree output lands on one
                # contiguous mvbuf run). 22 R / 42 C balances DVE vs ACT.
                NC_G = [6, 5, 5, 5, 6, 5, 5, 5]
                for g in range(NGROUPS):
                    n_c = NC_G[g]
                    stage = tailp.tile([128, n_c * 8, 128], BF16, tag="st")
                    if g + 2 < NGROUPS:
                        emb_q(g + 2)
                    halves = [(4 * g + k // 2, k % 2) for k in range(8)]
                    r_list = halves[: 8 - n_c]
                    c_list = halves[8 - n_c :]
                    # emit C/R interleaved so ACT and DVE run concurrently
                    order = []
                    for k in range(n_c):
                        order.append((c_list[k], k))
                        if k < len(r_list):
                            order.append((r_list[k], None))
                    for (i, h), c_idx in order:
                        score_half(i, h, stage, c_idx)
                    tail_tree(g, stage, n_c)
                    apply_rq(g)
                    group_sums(g, logits_ps)

                out_sb = outp.tile([128, DOCS_PER_CORE], F32)
                nc.scalar.copy(out_sb[:], logits_ps[:])
                nc.sync.dma_start(out_d[:], out_sb[:])

    nc.compile()
    return nc


def _host_inputs(query_h, doc_h, W, b):
    """Shard + lay out inputs for the 8 cores (bf16 on the wire)."""
    import ml_dtypes

    bf16 = ml_dtypes.bfloat16
    qT = np.ascontiguousarray(
        query_h.reshape(NQ_TOK, H).T.reshape(2, 128, NQ_TOK).transpose(1, 0, 2)
    ).astype(bf16)
    gpad = np.zeros((128, 256), np.float32)
    for s in range(128):
        gpad[s, 124 + s // LQ] = 1.0
    consts = np.concatenate(
        [
            W[:128],
            W[128:],
            np.ones((128, 128), np.float32),
            gpad,
        ],
        axis=1,
    ).astype(bf16)
    common = {
        "qhT": qT,
        "consts": consts,
        "bb": b.reshape(128, 1).astype(np.float32),
    }
    in_maps = []
    for k in range(NCORES):
        dT = np.ascontiguousarray(
            doc_h[k * DOCS_PER_CORE : (k + 1) * DOCS_PER_CORE]
            .reshape(ND_TOK, H)
            .T.reshape(2, 128, ND_TOK)
            .transpose(1, 0, 2)
        ).astype(bf16)
        in_maps.append({**common, "dhT": dT})
    return in_maps


_PROGRAM = None


def _get_program() -> bass.Bass:
    global _PROGRAM
    if _PROGRAM is None:
        _PROGRAM = _build_program()
    return _PROGRAM


class _Runner:
    """Caches the sharded jitted executable so repeat calls skip rebuild.

    Mirrors bass2jax.run_bass_via_pjrt's multi-core branch: inputs for the 8
    cores are concatenated on axis 0 and shard_mapped over a 1-D core mesh,
    with pre-zeroed donated output buffers.
    """

    def __init__(self):
        import jax
        import numpy as _np
        from jax.sharding import Mesh, PartitionSpec
        from jax.experimental.shard_map import shard_map
        from concourse import bass2jax, mybir as _mb

        bass2jax.install_neuronx_cc_hook()
        nc = _get_program()
        self.nc = nc

        partition_name = (
            nc.partition_id_tensor.name if nc.partition_id_tensor else None
        )
        in_names, out_names, out_avals, zero_outs = [], [], [], []
        for alloc in nc.m.functions[0].allocations:
            if not isinstance(alloc, _mb.MemoryLocationSet):
                continue
            name = alloc.memorylocations[0].name
            if alloc.kind == "ExternalInput":
                if name != partition_name:
                    in_names.append(name)
            elif alloc.kind == "ExternalOutput":
                shape = tuple(alloc.tensor_shape)
                dt_np = _mb.dt.np(alloc.dtype)
                out_names.append(name)
                out_avals.append(jax.core.ShapedArray(shape, dt_np))
                zero_outs.append(_np.zeros(shape, dt_np))

        n_params = len(in_names)
        n_outs = len(out_names)
        all_in_names = list(in_names) + list(out_names)
        if partition_name is not None:
            all_in_names.append(partition_name)

        def _body(*args):
            operands = list(args)
            if partition_name is not None:
                operands.append(bass2jax.partition_id_tensor())
            outs = bass2jax._bass_exec_p.bind(
                *operands,
                out_avals=tuple(out_avals),
                in_names=tuple(all_in_names),
                out_names=tuple(out_names),
                lowering_input_output_aliases=(),
                sim_require_finite=True,
                sim_require_nnan=True,
                nc=nc,
            )
            return tuple(outs)

        devices = jax.devices()[:NCORES]
        mesh = Mesh(np.asarray(devices), ("core",))
        in_specs = (PartitionSpec("core"),) * (n_params + n_outs)
        out_specs = (PartitionSpec("core"),) * n_outs
        self._fn = jax.jit(
            shard_map(
                _body,
                mesh=mesh,
                in_specs=in_specs,
                out_specs=out_specs,
                check_rep=False,
            ),
            donate_argnums=tuple(range(n_params, n_params + n_outs)),
            keep_unused=True,
        )
        self.in_names = in_names
        self.out_names = out_names
        self.out_avals = out_avals
        self.zero_outs = zero_outs
        self.n_params = n_params

    def concat_inputs(self, in_maps):
        return [
            np.concatenate([np.asarray(m[name]) for m in in_maps], axis=0)
            for name in self.in_names
        ]

    def concat_zeros(self):
        return [
            np.zeros((NCORES * z.shape[0], *z.shape[1:]), z.dtype)
            for z in self.zero_outs
        ]

    def run(self, concat_in):
        out_arrs = self._fn(*concat_in, *self.concat_zeros())
        return out_arrs

    def results(self, out_arrs):
        return [
            {
                name: np.asarray(out_arrs[i]).reshape(
                    NCORES, *self.out_avals[i].shape
                )[c]
                for i, name in enumerate(self.out_names)
            }
            for c in range(NCORES)
        ]


_RUNNER = None


def _get_runner() -> "_Runner":
    global _RUNNER
    if _RUNNER is None:
        _RUNNER = _Runner()
    return _RUNNER


def kernel(query_h, doc_h, W, b):
    query_h = np.asarray(query_h, np.float32)
    doc_h = np.asarray(doc_h, np.float32)
    W = np.asarray(W, np.float32)
    b = np.asarray(b, np.float32)

    in_maps = _host_inputs(query_h, doc_h, W, b)
    runner = _get_runner()
    outs = runner.results(runner.run(runner.concat_inputs(in_maps)))
    return np.concatenate(
        [outs[k]["logits"] for k in range(NCORES)], axis=1
    ).astype(np.float32)


def bench(query_h, doc_h, W, b, iters=20):
    """Repeat-execute timing with device-resident inputs. Returns times (s)."""
    import time
    import jax

    in_maps = _host_inputs(
        np.asarray(query_h, np.float32),
        np.asarray(doc_h, np.float32),
        np.asarray(W, np.float32),
        np.asarray(b, np.float32),
    )
    runner = _get_runner()
    concat_in = [jax.device_put(a) for a in runner.concat_inputs(in_maps)]
    # warmup (also triggers compile)
    jax.block_until_ready(runner.run(concat_in))
    times = []
    for _ in range(iters):
        t0 = time.perf_counter()
        jax.block_until_ready(runner.run(concat_in))
        times.append(time.perf_counter() - t0)
    return times


# revision 18
# speedup vs baseline: 1.0485x; 1.0485x over previous
"""ColBERT-style max-sim retrieval kernel for 8 trn2 NeuronCores.

Computes, for query_h [Bq=128, Lq=32, H=256], doc_h [Bd=128, Ld=128, H=256],
W [256, 128], b [128]:

    q = l2norm(query_h @ W + b)          # [Bq, Lq, D=128]
    d = l2norm(doc_h  @ W + b)           # [Bd, Ld, D]
    logits[q, b] = sum_s max_t <q[q,s], d[b,t]>    # [Bq, Bd]

Sharding: docs split 8 x 16 across cores (queries replicated); each core
computes a [128, 16] column block of the logits; host concatenates.

Key structure (vs the 90.6us baseline, which was DVE-reduce-bound at 87%):
  - Everything ships/computes in bf16 (1 cycle/row matmuls, half the DMA).
  - Query normalization is algebraically moved past the max:
        sum_s max_t <q_s, d_t>/|q_s| = sum_s (1/|q_s|) max_t <q_s, d_t>
    so the q-side rrep/normalize (full [128, 4096] elementwise work)
    becomes a broadcast multiply on the tiny [128, 32, 16] max-values.
    |q_s|^2 comes from tiny per-tile PE matmuls (sq-tile @ ones-column)
    that land per-token sums on PARTITIONS (matching the score layout).
  - The max over doc tokens runs as a tensor_tensor max TREE instead of
    reduce_max: InstTensorReduce has no DVE fast modes (1 elem/lane/cyc)
    while TT-max consumes 2 inputs/lane/cycle and gets a further 2x on
    packed bf16 in SBUF. Level 1 (PSUM fp32 -> SBUF bf16) is split across
    Pool (~7/8, 427ns/half) and DVE (~1/8, 658ns/half); levels 2..7 run
    batched per 4-tile group ([128, 64, 64] -> [128, 64]) on DVE bf16.
  - Doc embeddings are normalized as in the baseline (the 1/|d_t| scale
    sits inside the max and cannot move out).
"""

import sys

import numpy as np

if "/opt/trn_rl_repo" not in sys.path:
    sys.path.insert(0, "/opt/trn_rl_repo")

import concourse.bass as bass
import concourse.tile as tile
from concourse import bacc, mybir
from concourse.bass_utils import run_bass_kernel_spmd

F32 = mybir.dt.float32
F32R = mybir.dt.float32r
BF16 = mybir.dt.bfloat16
AX = mybir.AxisListType
ALU = mybir.AluOpType
ACTF = mybir.ActivationFunctionType

# Problem constants (hardcoded per the harness contract).
BQ, LQ, BD, LD, H, D = 128, 32, 128, 128, 256, 128
NCORES = 8
DOCS_PER_CORE = BD // NCORES          # 16
NQ_TOK = BQ * LQ                      # 4096 query tokens (replicated)
ND_TOK = DOCS_PER_CORE * LD           # 2048 doc tokens per core
CHUNK = 512                           # embedding-phase token chunk (1 psum bank)
QS_TILES = NQ_TOK // 128              # 32 score row-tiles
GQ = 128 // LQ                        # 4 queries per qs-tile
NGROUPS = QS_TILES // 4               # 8 tail-tree groups (4 tiles each)

# consts layout (bf16): W0 | W1 | ones | gpad
NCONST = 128 + 128 + 128 + 256


def _build_program() -> bass.Bass:
    # Bacc (not plain Bass): its compile() runs move_matmul_waits_to_ldweights
    # and generate_event_semaphores, which split multi-wait matmuls into
    # event-semaphore helpers -- walrus rejects a fused matmul with >1 wait.
    nc = bacc.Bacc("TRN2", target_bir_lowering=False)

    # Inputs: both H-halves packed per tensor so one DMA delivers a chunk.
    qhT = nc.dram_tensor("qhT", [128, 2, NQ_TOK], BF16, kind="ExternalInput")
    dhT = nc.dram_tensor("dhT", [128, 2, ND_TOK], BF16, kind="ExternalInput")
    consts = nc.dram_tensor("consts", [128, NCONST], BF16, kind="ExternalInput")
    bb = nc.dram_tensor("bb", [128, 1], F32, kind="ExternalInput")
    out_d = nc.dram_tensor("logits", [128, DOCS_PER_CORE], F32, kind="ExternalOutput")

    with tile.TileContext(nc) as tc:
        with (
            tc.tile_pool(name="consts", bufs=1) as constp,
            tc.tile_pool(name="inputs", bufs=1) as inp,
            tc.tile_pool(name="embs", bufs=1) as embp,
        ):
            consts_sb = constp.tile([128, NCONST], BF16)
            b_sb = constp.tile([128, 1], F32)
            nc.sync.dma_start(consts_sb[:], consts[:])
            nc.sync.dma_start(b_sb[:], bb[:])
            w0_sb = consts_sb[:, 0:128]
            w1_sb = consts_sb[:, 128:256]
            ones_sb = consts_sb[:, 256:384]
            gpad_sb = consts_sb[:, 384:640]

            dhT_sb = inp.tile([128, 2, ND_TOK], BF16)
            qhT_sb = inp.tile([128, 2, NQ_TOK], BF16)
            # Docs first (they gate every score); q0 rides between d1 and d2.
            for c in range(0, 2 * CHUNK, CHUNK):
                nc.sync.dma_start(dhT_sb[:, :, c : c + CHUNK], dhT[:, :, c : c + CHUNK])
            nc.sync.dma_start(qhT_sb[:, :, 0:CHUNK], qhT[:, :, 0:CHUNK])
            for c in range(2 * CHUNK, ND_TOK, CHUNK):
                nc.sync.dma_start(dhT_sb[:, :, c : c + CHUNK], dhT[:, :, c : c + CHUNK])
            for c in range(CHUNK, NQ_TOK, CHUNK):
                nc.sync.dma_start(qhT_sb[:, :, c : c + CHUNK], qhT[:, :, c : c + CHUNK])

            embq = embp.tile([128, NQ_TOK], BF16)   # q emb + b, UNnormalized [D, tok]
            embd = embp.tile([128, ND_TOK], BF16)   # normalized d emb [D, tok]
            mvbuf = embp.tile([128, QS_TILES, DOCS_PER_CORE], BF16)  # raw maxes
            mvsc = embp.tile([128, QS_TILES, DOCS_PER_CORE], BF16)   # rq-scaled
            rq = embp.tile([128, QS_TILES], F32R)   # 1/|q_s| per tile-token

            with (
                tc.tile_pool(name="pe_psum", bufs=2, space="PSUM") as pep,
                tc.tile_pool(name="sc_psum", bufs=2, space="PSUM") as scp,
                tc.tile_pool(name="ss_psum", bufs=1, space="PSUM") as ssp,
                tc.tile_pool(name="sm_psum", bufs=1, space="PSUM") as smp,
                tc.tile_pool(name="actwork", bufs=2) as actp,
                tc.tile_pool(name="tails", bufs=2) as tailp,
                tc.tile_pool(name="outp", bufs=1) as outp,
            ):
                # Absorb DMA sem waits ahead of fused matmuls (single-wait
                # restriction): a tiny self-referencing observer matmul.
                def pe_observe(x):
                    ob = pep.tile([1, 2], F32, tag="pe")
                    nc.tensor.matmul(
                        ob[:], x[:, 0:1], x[:, 0:2], start=True, stop=True
                    )

                pe_observe(consts_sb)

                # First activation = Abs_reciprocal_sqrt so the table-load
                # pass picks abs_reciprocal_sqrt_and_small (contains square,
                # copy, identity too): exactly one ACT_TABLE_LOAD overall.
                act_seed = actp.tile([128, 1], F32, tag="seed", bufs=1)
                nc.scalar.activation(
                    act_seed[:], b_sb[:, 0:1], ACTF.Abs_reciprocal_sqrt
                )

                # SBUF logits accumulator (PSUM is fully booked; group sums
                # land in a recycled 1-bank tile and are TT-added here).
                acc_sb = outp.tile([128, DOCS_PER_CORE], F32)

                def emb_doc(ci):
                    """Project+normalize doc tokens [ci*CHUNK, ...) -> embd."""
                    c = ci * CHUNK
                    pe = pep.tile([128, CHUNK], F32, tag="pe")
                    nc.tensor.matmul(
                        pe[:], w0_sb[:], dhT_sb[:, 0, c : c + CHUNK],
                        start=True, stop=False,
                    )
                    nc.tensor.matmul(
                        pe[:], w1_sb[:], dhT_sb[:, 1, c : c + CHUNK],
                        start=False, stop=True,
                    )
                    sq = actp.tile([128, CHUNK], BF16, tag="sq")
                    nc.scalar.activation(sq[:], pe[:], ACTF.Square, bias=b_sb[:])
                    # Cross-partition sum of squares broadcast to all rows.
                    ss = ssp.tile([128, CHUNK], F32, tag="ss")
                    nc.tensor.matmul(ss[:], ones_sb[:], sq[:], start=True, stop=True)
                    rrep = actp.tile([128, CHUNK], F32R, tag="rrep")
                    nc.scalar.activation(rrep[:], ss[:], ACTF.Abs_reciprocal_sqrt)
                    # embd = (pe + b) * rrep  (bf16 on write)
                    nc.vector.scalar_tensor_tensor(
                        out=embd[:, c : c + CHUNK],
                        in0=pe[:],
                        scalar=b_sb[:],
                        in1=rrep[:],
                        op0=ALU.add,
                        op1=ALU.mult,
                    )

                def emb_q(ci):
                    """Project q chunk ci; emit embq (+b), sqq, ssq cols, rq."""
                    c = ci * CHUNK
                    pe = pep.tile([128, CHUNK], F32, tag="pe")
                    nc.tensor.matmul(
                        pe[:], w0_sb[:], qhT_sb[:, 0, c : c + CHUNK],
                        start=True, stop=False,
                    )
                    nc.tensor.matmul(
                        pe[:], w1_sb[:], qhT_sb[:, 1, c : c + CHUNK],
                        start=False, stop=True,
                    )
                    nc.scalar.activation(
                        embq[:, c : c + CHUNK], pe[:], ACTF.Identity, bias=b_sb[:]
                    )
                    # sqq = embq^2 on Pool (SBUF-only mult is legal there;
                    # frees ACT for the PSUM score-copy pipeline).
                    sqq = actp.tile([128, CHUNK], BF16, tag="sqq")
                    nc.gpsimd.tensor_tensor(
                        out=sqq[:],
                        in0=embq[:, c : c + CHUNK],
                        in1=embq[:, c : c + CHUNK],
                        op=ALU.mult,
                    )
                    # Per-token sum over D (partitions): land tokens on
                    # partitions via tiny matmuls, one column per qs-tile.
                    # One accumulation-group start per chunk: a second
                    # start=True in the same bank re-marks the whole 2KB
                    # zero-region, which would pending-zero the earlier
                    # columns on hardware before the rsqrt reads them. The
                    # small tile is recycled (shared with the group-sum
                    # tiles), so cross-use ordering rides pool recycling.
                    ssq_ps = smp.tile([128, DOCS_PER_CORE], F32, tag="sm")
                    for j in range(4):
                        nc.tensor.matmul(
                            ssq_ps[:, j : j + 1],
                            sqq[:, j * 128 : (j + 1) * 128],
                            ones_sb[:, 0:1],
                            start=(j == 0),
                            stop=(j == 3),
                            skip_group_check=True,
                        )
                    nc.scalar.activation(
                        rq[:, 4 * ci : 4 * ci + 4],
                        ssq_ps[:, 0:4],
                        ACTF.Abs_reciprocal_sqrt,
                    )

                def score_half(i, h, stage, c_idx):
                    """Scores for qs-tile i, docs [8h, 8h+8).

                    No engine may read two non-scalar PSUM operands in one
                    instruction (and Pool has no max op at all), so the
                    [128, 8, 128] PSUM scores leave PSUM through one of the
                    two legal 1-elem/lane/cycle consumers:
                      - R-halves: DVE reduce_max straight to mvbuf
                      - C-halves: ACT Copy -> bf16 stage; DVE then runs the
                        batched bf16 max tree at 2 max-ops/lane/cycle.
                    """
                    qsl = embq[:, i * 128 : (i + 1) * 128]
                    sc = scp.tile([128, 8, 128], F32, tag="sc")
                    for j in range(2):
                        col = h * 1024 + j * 512
                        nc.tensor.matmul(
                            sc[:, j * 4 : (j + 1) * 4, :],
                            qsl,
                            embd[:, col : col + 512],
                            start=True,
                            stop=True,
                        )
                    if c_idx is None:
                        nc.vector.reduce_max(
                            mvbuf[:, i, h * 8 : (h + 1) * 8], sc[:], axis=AX.X
                        )
                    else:
                        nc.scalar.activation(
                            stage[:, c_idx * 8 : c_idx * 8 + 8, :], sc[:],
                            ACTF.Copy,
                        )

                def tail_tree(g, stage, n_c):
                    """DVE bf16 max tree over the n_c ACT-copied halves of
                    group g -> the tail of mvbuf's flat [tile, doc] range."""
                    m = n_c * 8
                    t1 = tailp.tile([128, m, 64], BF16, tag="t1")
                    nc.vector.tensor_tensor(
                        out=t1[:], in0=stage[:, :, 0:64], in1=stage[:, :, 64:128],
                        op=ALU.max,
                    )
                    t2 = tailp.tile([128, m, 32], BF16, tag="t2")
                    nc.vector.tensor_tensor(
                        out=t2[:], in0=t1[:, :, 0:32], in1=t1[:, :, 32:64], op=ALU.max
                    )
                    t3 = tailp.tile([128, m, 16], BF16, tag="t3")
                    nc.vector.tensor_tensor(
                        out=t3[:], in0=t2[:, :, 0:16], in1=t2[:, :, 16:32], op=ALU.max
                    )
                    t4 = tailp.tile([128, m, 8], BF16, tag="t4")
                    nc.vector.tensor_tensor(
                        out=t4[:], in0=t3[:, :, 0:8], in1=t3[:, :, 8:16], op=ALU.max
                    )
                    t5 = tailp.tile([128, m, 4], BF16, tag="t5")
                    nc.vector.tensor_tensor(
                        out=t5[:], in0=t4[:, :, 0:4], in1=t4[:, :, 4:8], op=ALU.max
                    )
                    t6 = tailp.tile([128, m, 2], BF16, tag="t6")
                    nc.vector.tensor_tensor(
                        out=t6[:], in0=t5[:, :, 0:2], in1=t5[:, :, 2:4], op=ALU.max
                    )
                    start = (4 * g + 4) * 16 - m
                    mv = mvbuf[:, :, :].rearrange("p a b -> p (a b)")[
                        :, start : start + m
                    ].unsqueeze(2)
                    nc.vector.tensor_tensor(
                        out=mv, in0=t6[:, :, 0:1], in1=t6[:, :, 1:2], op=ALU.max
                    )

                def apply_rq(g):
                    rqb = rq[:, 4 * g : 4 * g + 4, None].broadcast_to(
                        [128, 4, DOCS_PER_CORE]
                    )
                    nc.vector.tensor_tensor(
                        out=mvsc[:, 4 * g : 4 * g + 4, :],
                        in0=mvbuf[:, 4 * g : 4 * g + 4, :],
                        in1=rqb,
                        op=ALU.mult,
                    )

                def group_sums(g):
                    # Sum over the 32 tokens of each query via a sliding
                    # block-diagonal window of gpad (weights are 0/1), into a
                    # recycled PSUM tile; accumulate into acc_sb on DVE.
                    lg = smp.tile([128, DOCS_PER_CORE], F32, tag="sm")
                    for i in range(4 * g, 4 * g + 4):
                        off = 124 - GQ * i
                        nc.tensor.matmul(
                            lg[:],
                            gpad_sb[:, off : off + 128],
                            mvsc[:, i, :],
                            start=(i % 4 == 0),
                            stop=(i % 4 == 3),
                            skip_group_check=True,
                        )
                    if g == 0:
                        nc.vector.tensor_copy(acc_sb[:], lg[:])
                    else:
                        nc.vector.tensor_tensor(
                            out=acc_sb[:], in0=lg[:], in1=acc_sb[:], op=ALU.add
                        )

                # --- ramp ---
                emb_doc(0)
                emb_doc(1)
                emb_q(0)
                emb_doc(2)
                emb_q(1)
                emb_doc(3)

                # --- steady state: 8 groups of 4 qs-tiles ---
                # R/C split per group (R-halves head, C-halves tail of the
                # group's flat [tile, h] order so the tree output lands on one
                # contiguous mvbuf run). 20 R / 44 C balances DVE vs ACT.
                # The tree for group g-1 is emitted AFTER group g's halves so
                # DVE's reduces fill the wait for ACT's copies (software
                # pipelining: tree runs one group behind).
                NC_G = [6, 5, 6, 5, 6, 5, 6, 5]
                stages = {}

                def finish_group(g):
                    tail_tree(g, stages.pop(g), NC_G[g])
                    apply_rq(g)
                    group_sums(g)

                for g in range(NGROUPS):
                    n_c = NC_G[g]
                    stage = tailp.tile([128, n_c * 8, 128], BF16, tag="st")
                    stages[g] = stage
                    if g + 2 < NGROUPS:
                        emb_q(g + 2)
                    halves = [(4 * g + k // 2, k % 2) for k in range(8)]
                    r_list = halves[: 8 - n_c]
                    c_list = halves[8 - n_c :]
                    # emit C/R interleaved so ACT and DVE run concurrently
                    order = []
                    for k in range(n_c):
                        order.append((c_list[k], k))
                        if k < len(r_list):
                            order.append((r_list[k], None))
                    for (i, h), c_idx in order:
                        score_half(i, h, stage, c_idx)
                    if g >= 1:
                        finish_group(g - 1)
                finish_group(NGROUPS - 1)

                nc.sync.dma_start(out_d[:], acc_sb[:])

    nc.compile()
    return nc


def _host_inputs(query_h, doc_h, W, b):
    """Shard + lay out inputs for the 8 cores (bf16 on the wire)."""
    import ml_dtypes

    bf16 = ml_dtypes.bfloat16
    qT = np.ascontiguousarray(
        query_h.reshape(NQ_TOK, H).T.reshape(2, 128, NQ_TOK).transpose(1, 0, 2)
    ).astype(bf16)
    gpad = np.zeros((128, 256), np.float32)
    for s in range(128):
        gpad[s, 124 + s // LQ] = 1.0
    consts = np.concatenate(
        [
            W[:128],
            W[128:],
            np.ones((128, 128), np.float32),
            gpad,
        ],
        axis=1,
    ).astype(bf16)
    common = {
        "qhT": qT,
        "consts": consts,
        "bb": b.reshape(128, 1).astype(np.float32),
    }
    in_maps = []
    for k in range(NCORES):
        dT = np.ascontiguousarray(
            doc_h[k * DOCS_PER_CORE : (k + 1) * DOCS_PER_CORE]
            .reshape(ND_TOK, H)
            .T.reshape(2, 128, ND_TOK)
            .transpose(1, 0, 2)
        ).astype(bf16)
        in_maps.append({**common, "dhT": dT})
    return in_maps


_PROGRAM = None


def _get_program() -> bass.Bass:
    global _PROGRAM
    if _PROGRAM is None:
        _PROGRAM = _build_program()
    return _PROGRAM


class _Runner:
    """Caches the sharded jitted executable so repeat calls skip rebuild.

    Mirrors bass2jax.run_bass_via_pjrt's multi-core branch: inputs for the 8
    cores are concatenated on axis 0 and shard_mapped over a 1-D core mesh,
    with pre-zeroed donated output buffers.
    """

    def __init__(self):
        import jax
        import numpy as _np
        from jax.sharding import Mesh, PartitionSpec
        from jax.experimental.shard_map import shard_map
        from concourse import bass2jax, mybir as _mb

        bass2jax.install_neuronx_cc_hook()
        nc = _get_program()
        self.nc = nc

        partition_name = (
            nc.partition_id_tensor.name if nc.partition_id_tensor else None
        )
        in_names, out_names, out_avals, zero_outs = [], [], [], []
        for alloc in nc.m.functions[0].allocations:
            if not isinstance(alloc, _mb.MemoryLocationSet):
                continue
            name = alloc.memorylocations[0].name
            if alloc.kind == "ExternalInput":
                if name != partition_name:
                    in_names.append(name)
            elif alloc.kind == "ExternalOutput":
                shape = tuple(alloc.tensor_shape)
                dt_np = _mb.dt.np(alloc.dtype)
                out_names.append(name)
                out_avals.append(jax.core.ShapedArray(shape, dt_np))
                zero_outs.append(_np.zeros(shape, dt_np))

        n_params = len(in_names)
        n_outs = len(out_names)
        all_in_names = list(in_names) + list(out_names)
        if partition_name is not None:
            all_in_names.append(partition_name)

        def _body(*args):
            operands = list(args)
            if partition_name is not None:
                operands.append(bass2jax.partition_id_tensor())
            outs = bass2jax._bass_exec_p.bind(
                *operands,
                out_avals=tuple(out_avals),
                in_names=tuple(all_in_names),
                out_names=tuple(out_names),
                lowering_input_output_aliases=(),
                sim_require_finite=True,
                sim_require_nnan=True,
                nc=nc,
            )
            return tuple(outs)

        devices = jax.devices()[:NCORES]
        mesh = Mesh(np.asarray(devices), ("core",))
        in_specs = (PartitionSpec("core"),) * (n_params + n_outs)
        out_specs = (PartitionSpec("core"),) * n_outs
        self._fn = jax.jit(
            shard_map(
                _body,
                mesh=mesh,
                in_specs=in_specs,
                out_specs=out_specs,
                check_rep=False,
            ),
            donate_argnums=tuple(range(n_params, n_params + n_outs)),
            keep_unused=True,
        )
        self.in_names = in_names
        self.out_names = out_names
        self.out_avals = out_avals
        self.zero_outs = zero_outs
        self.n_params = n_params

    def concat_inputs(self, in_maps):
        return [
            np.concatenate([np.asarray(m[name]) for m in in_maps], axis=0)
            for name in self.in_names
        ]

    def concat_zeros(self):
        return [
            np.zeros((NCORES * z.shape[0], *z.shape[1:]), z.dtype)
            for z in self.zero_outs
        ]

    def run(self, concat_in):
        out_arrs = self._fn(*concat_in, *self.concat_zeros())
        return out_arrs

    def results(self, out_arrs):
        return [
            {
                name: np.asarray(out_arrs[i]).reshape(
                    NCORES, *self.out_avals[i].shape
                )[c]
                for i, name in enumerate(self.out_names)
            }
            for c in range(NCORES)
        ]


_RUNNER = None


def _get_runner() -> "_Runner":
    global _RUNNER
    if _RUNNER is None:
        _RUNNER = _Runner()
    return _RUNNER


def kernel(query_h, doc_h, W, b):
    query_h = np.asarray(query_h, np.float32)
    doc_h = np.asarray(doc_h, np.float32)
    W = np.asarray(W, np.float32)
    b = np.asarray(b, np.float32)

    in_maps = _host_inputs(query_h, doc_h, W, b)
    runner = _get_runner()
    outs = runner.results(runner.run(runner.concat_inputs(in_maps)))
    return np.concatenate(
        [outs[k]["logits"] for k in range(NCORES)], axis=1
    ).astype(np.float32)


def bench(query_h, doc_h, W, b, iters=20):
    """Repeat-execute timing with device-resident inputs. Returns times (s)."""
    import time
    import jax

    in_maps = _host_inputs(
        np.asarray(query_h, np.float32),
        np.asarray(doc_h, np.float32),
        np.asarray(W, np.float32),
        np.asarray(b, np.float32),
    )
    runner = _get_runner()
    concat_in = [jax.device_put(a) for a in runner.concat_inputs(in_maps)]
    # warmup (also triggers compile)
    jax.block_until_ready(runner.run(concat_in))
    times = []
    for _ in range(iters):
        t0 = time.perf_counter()
        jax.block_until_ready(runner.run(concat_in))
        times.append(time.perf_counter() - t0)
    return times
